# revision 19
# baseline (speedup 1.0000x reference)
"""Trainium2 Bass kernel: cached-SDDMM linear (top-k masked matvec).

Math: out = weight @ (x masked to its top-1228 |x| entries) + bias.

Strategy (8-core tensor parallel, sharded along out_features):
  - Each core holds a [1792, 4096] row-shard of weight and the full x.
  - Device-side exact top-k threshold via 3 chained gpsimd kth_largest
    calls (heap cap is 510 < 1228, so select rank 512, mask, rank 1024,
    mask, rank 1228) -- each stage returns an exact order statistic.
  - Mask x on-chip, broadcast across partitions, then stream the weight
    shard in natural row-major layout: per 128-row tile, DVE multiplies
    by the masked-x broadcast and ScalarE reduces along the free dim via
    activation's accumulate output (the custom fused DVE reduce op is
    not executable on this runtime).  Memory-bound: the full weight
    shard (29.4 MB/core) streams through SBUF exactly once.
"""

import numpy as np

import concourse.tile as tile
from concourse import bacc, mybir
from concourse.bass_utils import run_bass_kernel_spmd

N_CORES = 8
IN_F = 4096
OUT_F = 14336
ROWS_PER_CORE = OUT_F // N_CORES  # 1792
M_TILES = ROWS_PER_CORE // 128  # 14
N_PER_LANE = IN_F // 128  # 32

# Exact top-1228 threshold in three kth_largest stages (heap cap 510).
# Between stages, already-selected elements are zeroed (not -inf), so
# n_valid stays 4096 and the quantiles are compile-time exact:
#   A: desc[511]  = rank 512   (k_adj 510)
#   B: desc2[511] = rank 1024  (k_adj 510; ranks 1..512 now sort last)
#   C: desc3[203] = rank 1228  (k_adj 202)
STAGES = [
    (510, 1.0 - 0.12466),  # floor(0.12466*4095) = 510
    (510, 1.0 - 0.12466),  # floor(0.12466*4095) = 510
    (250, 1.0 - 0.04945),  # floor(0.04945*4095) = 202
]

_PROGRAM = None


def _build_program():
    f32 = mybir.dt.float32
    nc = bacc.Bacc(
        "TRN2", target_bir_lowering=False, debug=False, num_devices=N_CORES
    )
    w_d = nc.dram_tensor("w", [ROWS_PER_CORE, IN_F], f32, kind="ExternalInput").ap()
    xb_d = nc.dram_tensor("xb", [128, IN_F], f32, kind="ExternalInput").ap()
    xs_d = nc.dram_tensor("xs", [128, N_PER_LANE], f32, kind="ExternalInput").ap()
    b_d = nc.dram_tensor("b", [128, M_TILES], f32, kind="ExternalInput").ap()
    o_d = nc.dram_tensor("o", [128, M_TILES], f32, kind="ExternalOutput").ap()

    with tile.TileContext(nc) as tc:
        with (
            tc.tile_pool(name="persist", bufs=1) as persist,
            tc.tile_pool(name="small", bufs=1) as small,
            tc.tile_pool(name="wpool", bufs=6) as wpool,
        ):
            xb = persist.tile([128, IN_F], f32)
            xabs = persist.tile([128, IN_F], f32)
            xmb = persist.tile([128, IN_F], f32)
            bias = small.tile([128, M_TILES], f32)
            res = small.tile([128, M_TILES], f32)
            xs = small.tile([128, N_PER_LANE], f32)

            nc.sync.dma_start(out=xs, in_=xs_d)
            nc.sync.dma_start(out=xb, in_=xb_d)
            nc.sync.dma_start(out=bias, in_=b_d)

            # |x| in both layouts on ScalarE (AluOpType.abs_max is
            # simulator-only; the Abs activation is real hardware)
            xas = small.tile([128, N_PER_LANE], f32)
            nc.scalar.activation(xas, xs, mybir.ActivationFunctionType.Abs)
            nc.scalar.activation(xabs, xb, mybir.ActivationFunctionType.Abs)

            # 3-stage exact selection of the 1228th-largest |x|
            cur = xas
            thr = None
            for si, (heap_k, q) in enumerate(STAGES):
                kout = small.tile([1, 2], f32, tag=f"kout{si}")
                nc.gpsimd.kth_largest(
                    kout, cur, n_per_lane=N_PER_LANE, k=heap_k, quantile=q
                )
                thr = small.tile([128, 1], f32, tag=f"thr{si}")
                nc.gpsimd.partition_broadcast(thr, kout[0:1, 1:2], channels=128)
                if si < len(STAGES) - 1:
                    # zero out everything >= this stage's threshold
                    nxt = small.tile([128, N_PER_LANE], f32, tag=f"xas{si + 1}")
                    nc.vector.scalar_tensor_tensor(
                        out=nxt,
                        in0=cur,
                        scalar=thr,
                        in1=cur,
                        op0=mybir.AluOpType.is_lt,
                        op1=mybir.AluOpType.mult,
                    )
                    cur = nxt

            # xmb[p, i] = x[i] if |x[i]| >= t* else 0
            nc.vector.scalar_tensor_tensor(
                out=xmb,
                in0=xabs,
                scalar=thr,
                in1=xb,
                op0=mybir.AluOpType.is_ge,
                op1=mybir.AluOpType.mult,
            )

            # main loop: per 128-row tile, DVE multiplies W by the
            # masked-x broadcast, ScalarE sums the products along the
            # free dim into one accumulator column.
            with tc.tile_pool(name="prod", bufs=2) as ppool:
                for m in range(M_TILES):
                    wt = wpool.tile([128, IN_F], f32, tag="w")
                    nc.sync.dma_start(
                        out=wt, in_=w_d[m * 128 : (m + 1) * 128, :]
                    )
                    prod = ppool.tile([128, IN_F], f32, tag="prod")
                    nc.vector.tensor_mul(prod, wt, xmb)
                    nc.scalar.activation(
                        prod,
                        prod,
                        mybir.ActivationFunctionType.Copy,
                        accum_out=res[:, m : m + 1],
                    )

            out_j = small.tile([128, M_TILES], f32)
            nc.vector.tensor_add(out_j, res, bias)
            nc.sync.dma_start(out=o_d, in_=out_j)

    # Bacc's compile lowers multi-sem waits (the TPB instruction
    # encodings fit only one), auto-inserts the gpsimd library loads,
    # and runs the extended-instruction ISA codegen.
    nc.compile()
    return nc


def _get_program():
    global _PROGRAM
    if _PROGRAM is None:
        _PROGRAM = _build_program()
    return _PROGRAM


def _make_in_maps(x, weight, bias):
    xf = np.ascontiguousarray(np.asarray(x, dtype=np.float32).reshape(-1))
    weight = np.asarray(weight, dtype=np.float32)
    bias = np.asarray(bias, dtype=np.float32)
    xb = np.ascontiguousarray(np.broadcast_to(xf, (128, IN_F)))
    xs = np.ascontiguousarray(xf.reshape(128, N_PER_LANE))
    in_maps = []
    for c in range(N_CORES):
        lo, hi = c * ROWS_PER_CORE, (c + 1) * ROWS_PER_CORE
        wsh = np.ascontiguousarray(weight[lo:hi])
        bsh = np.ascontiguousarray(bias[lo:hi].reshape(M_TILES, 128).T)
        in_maps.append({"w": wsh, "xb": xb, "xs": xs, "b": bsh})
    return in_maps


def _gather_out(per_core_outs):
    shards = [np.asarray(o).T.reshape(-1) for o in per_core_outs]
    return np.concatenate(shards).reshape(1, 1, OUT_F).astype(np.float32)


def kernel(x, weight, bias):
    nc = _get_program()
    in_maps = _make_in_maps(x, weight, bias)
    res = run_bass_kernel_spmd(nc, in_maps, list(range(N_CORES)))
    return _gather_out([res.results[c]["o"] for c in range(N_CORES)])
